# revision 35
# baseline (speedup 1.0000x reference)
"""Trainium2 kernel for nn_PolynomialLayer: out = [x, x_i*x_j (i<=j)] @ W.T + bias.

Data-parallel over batch across 8 NeuronCores. Each core:
  - receives x^T for its 1024-row batch shard ([128 feat, 1024 b]) plus 64
    partition-rotated copies (host np.roll; pure data movement),
  - builds the 8256 pairwise-product features on the vector engine as 65
    full-128-partition tensor_tensor multiplies (chunk d: xT * rot_d covers
    all index pairs with cyclic difference {d, 128-d}),
  - accumulates out^T[512, 1024] = sum_c Wc.T @ PTc on the tensor engine
    (66 K-chunks of 128, all 8 PSUM banks: 4 n-chunks x 2 b-chunks),
  - adds bias during the PSUM->SBUF copies (split scalar/vector engines).
The host pre-permutes/transposes the weight matrix so its column order
matches the on-chip feature-chunk layout.
"""

import os
import sys
import numpy as np

for _p in ("/opt/trn_rl_repo",):
    if os.path.isdir(_p) and _p not in sys.path:
        sys.path.append(_p)

B, D, NOUT = 8192, 128, 512
NCORES = 8
BC = B // NCORES            # 1024 batch rows per core
NCHUNK = 66                 # 1 linear + 1 squares + 64 rotation chunks
NROT = 64                   # rotation distances d=1..64
NB = BC // 512              # moving-operand chunks per core (2)
NN = NOUT // 128            # output partition chunks (4)

COMPUTE_DT = os.environ.get("POLY_COMPUTE_DT", "bfloat16")  # bfloat16 | mixed | float32r


def _ensure_axon_hooks_stub():
    """concourse's trace path imports antenv.axon_hooks; provide a stub if
    this image lacks it so an env-triggered trace degrades instead of
    crashing."""
    try:
        import antenv.axon_hooks  # noqa: F401
        return
    except Exception:
        pass
    try:
        import types
        import antenv
        m = types.ModuleType("antenv.axon_hooks")
        m._hook = None
        m.set_axon_ntff_profile_hook = lambda h: setattr(m, "_hook", h)
        m.get_axon_ntff_profile_hook = lambda: m._hook
        sys.modules["antenv.axon_hooks"] = m
        antenv.axon_hooks = m
    except Exception:
        pass


def _pair_index_map():
    """Map (chunk c, partition p) -> column index in the reference feature
    order (or -1 for padding).

    Reference order: [x_0..x_127] then pairs (i,j) i<=j in
    combinations_with_replacement order.
    Chunk layout: c=0 linear; c=1 squares; c=2..65 -> d=c-1 in 1..64 with
    (i,j) = sorted(p, (p+d) % 128); for d=64 only p<64 is valid.
    """
    idx = np.full((NCHUNK, D), -1, dtype=np.int64)
    off = 128 * np.arange(D) - (np.arange(D) * (np.arange(D) - 1)) // 2

    def pair_idx(i, j):
        return D + off[i] + (j - i)

    idx[0, :] = np.arange(D)
    p = np.arange(D)
    idx[1, :] = pair_idx(p, p)
    for d in range(1, NROT + 1):
        c = 1 + d
        q = (p + d) % D
        i = np.minimum(p, q)
        j = np.maximum(p, q)
        v = pair_idx(i, j)
        if d == NROT:
            v = np.where(p < 64, v, -1)
        idx[c, :] = v
    return idx


_nc_cache = None


def _build_nc():
    global _nc_cache
    if _nc_cache is not None:
        return _nc_cache
    import concourse.tile as tile
    from concourse import bacc, mybir

    # "mixed": f32r x/weights/products (precision), bf16 rotation streams (DMA)
    cdt = mybir.dt.float32r if COMPUTE_DT == "mixed" else getattr(mybir.dt, COMPUTE_DT)
    rdt = mybir.dt.bfloat16 if COMPUTE_DT == "mixed" else cdt
    nc = bacc.Bacc("TRN2", target_bir_lowering=False, debug=False)
    # partition-major DRAM layouts: one dma_start covers a GROUP of chunks
    # with large per-partition-contiguous descriptors.
    xT_ext = nc.dram_tensor("xT", [D, BC], cdt, kind="ExternalInput")
    rots_ext = nc.dram_tensor("rots", [D, NROT, BC], rdt, kind="ExternalInput")
    wp_ext = nc.dram_tensor("wp", [D, NCHUNK, NOUT], cdt, kind="ExternalInput")
    bias_ext = nc.dram_tensor("biasp", [D, NN], mybir.dt.float32, kind="ExternalInput")
    out_ext = nc.dram_tensor("out", [NOUT, BC], mybir.dt.float32, kind="ExternalOutput")

    # chunks per DMA group, small leading groups so the pipeline starts fast
    wg_sizes = [1, 1, 2, 4] + [6] * 9 + [4]          # sums to 66
    rg_sizes = [1, 1, 2, 4] + [6] * 9 + [2]          # sums to 64
    wg_starts = np.cumsum([0] + wg_sizes).tolist()
    rg_starts = np.cumsum([0] + rg_sizes).tolist()
    wg_of_chunk = {}
    for g, s in enumerate(wg_starts[:-1]):
        for c in range(s, wg_starts[g + 1]):
            wg_of_chunk[c] = g
    rg_of_rc = {}
    for g, s in enumerate(rg_starts[:-1]):
        for r in range(s, rg_starts[g + 1]):
            rg_of_rc[r] = g

    with tile.TileContext(nc) as tc:
        wide = COMPUTE_DT == "float32r"   # 4-byte rotations: tighter SBUF budget
        with (
            tc.tile_pool(name="xpool", bufs=1) as xpool,
            tc.tile_pool(name="wpool", bufs=4 if wide else 5) as wpool,
            tc.tile_pool(name="rpool", bufs=3 if wide else (4 if COMPUTE_DT == "mixed" else 6)) as rpool,
            tc.tile_pool(name="ppool", bufs=6 if COMPUTE_DT != "bfloat16" else 8) as ppool,
            tc.tile_pool(name="opool", bufs=1) as opool,
            tc.tile_pool(name="psum", bufs=1, space="PSUM") as psum,
        ):
            xT = xpool.tile([D, BC], cdt)
            nc.sync.dma_start(xT[:], xT_ext[:])

            ps = [[psum.tile([D, 512], mybir.dt.float32,
                             name=f"ps_{n}_{b}", tag=f"ps_{n}_{b}")
                   for b in range(NB)] for n in range(NN)]

            wg_tiles = {}
            rg_tiles = {}
            for c in range(NCHUNK):
                g = wg_of_chunk[c]
                if c == wg_starts[g]:
                    sz = wg_sizes[g]
                    wg = wpool.tile([D, sz * NOUT], cdt, name="wg", tag="wg")
                    nc.sync.dma_start(wg[:], wp_ext[:, c:c + sz, :])
                    wg_tiles[g] = wg
                rc = c - 2  # rotation index for this chunk
                if c >= 2:
                    rgi = rg_of_rc[rc]
                    if rc == rg_starts[rgi]:
                        sz = rg_sizes[rgi]
                        rg = rpool.tile([D, sz * BC], rdt, name="rg", tag="rg")
                        nc.sync.dma_start(rg[:], rots_ext[:, rc:rc + sz, :])
                        rg_tiles[rgi] = rg

                if c == 0:
                    mv = xT
                elif c == 1:
                    mv = ppool.tile([D, BC], cdt, name="pt", tag="pt")
                    nc.vector.tensor_mul(mv[:], xT[:], xT[:])
                else:
                    rg = rg_tiles[rg_of_rc[rc]]
                    roff = rc - rg_starts[rg_of_rc[rc]]
                    rslice = rg[:, roff * BC:(roff + 1) * BC]
                    mv = ppool.tile([D, BC], cdt, name="pt", tag="pt")
                    nc.vector.tensor_mul(mv[:], xT[:], rslice)
                wg = wg_tiles[g]
                woff = (c - wg_starts[g]) * NOUT
                for n in range(NN):
                    for b in range(NB):
                        nc.tensor.matmul(
                            ps[n][b][:],
                            wg[:, woff + n * 128:woff + (n + 1) * 128],
                            mv[:, b * 512:(b + 1) * 512],
                            start=(c == 0),
                            stop=(c == NCHUNK - 1),
                        )

            bias = xpool.tile([D, NN], mybir.dt.float32)
            nc.sync.dma_start(bias[:], bias_ext[:])
            obig = opool.tile([D, NN * NB * 512], mybir.dt.float32)
            for n in range(NN):
                for b in range(NB):
                    ot = obig[:, (n * NB + b) * 512:(n * NB + b + 1) * 512]
                    if b == 0:
                        nc.scalar.activation(
                            ot, ps[n][b][:],
                            mybir.ActivationFunctionType.Identity,
                            bias=bias[:, n:n + 1],
                        )
                    else:
                        nc.vector.tensor_scalar_add(ot, ps[n][b][:], bias[:, n:n + 1])
            nc.sync.dma_start(
                out_ext[:].rearrange("(n p) (b f) -> p n b f", n=NN, b=NB),
                obig[:].rearrange("p (n b f) -> p n b f", n=NN, b=NB),
            )

    nc.compile()
    _nc_cache = nc
    return nc


def _prep_inputs(x, weights, bias):
    if COMPUTE_DT == "bfloat16":
        import ml_dtypes
        cdt_np = np.dtype(ml_dtypes.bfloat16)
        rdt_np = cdt_np
    elif COMPUTE_DT == "mixed":
        import ml_dtypes
        cdt_np = np.dtype(np.float32)
        rdt_np = np.dtype(ml_dtypes.bfloat16)
    else:
        cdt_np = np.dtype(np.float32)
        rdt_np = cdt_np

    x = np.asarray(x, dtype=np.float32)
    weights = np.asarray(weights, dtype=np.float32)
    bias = np.asarray(bias, dtype=np.float32)

    idx = _pair_index_map()
    wcols = weights.T  # [8384, 512]
    wp = np.zeros((NCHUNK, D, NOUT), dtype=np.float32)
    valid = idx >= 0
    wp[valid] = wcols[idx[valid]]
    wp = np.ascontiguousarray(wp.transpose(1, 0, 2)).astype(cdt_np)  # [D, NCHUNK, NOUT]

    biasp = np.ascontiguousarray(bias.reshape(NN, 128).T)  # [128, NN] f32

    in_maps = []
    for k in range(NCORES):
        xs = np.ascontiguousarray(x[k * BC:(k + 1) * BC].T).astype(cdt_np)  # [128, BC]
        xr = xs.astype(rdt_np)
        rots = np.stack([np.roll(xr, -d, axis=0) for d in range(1, NROT + 1)])
        rots = rots.transpose(1, 0, 2)  # [D, NROT, BC] partition-major
        in_maps.append({
            "xT": xs,
            "rots": np.ascontiguousarray(rots),
            "wp": wp,
            "biasp": biasp,
        })
    return in_maps


def kernel(x, weights, bias):
    _ensure_axon_hooks_stub()
    from concourse.bass_utils import run_bass_kernel_spmd

    nc = _build_nc()
    in_maps = _prep_inputs(x, weights, bias)
    res = run_bass_kernel_spmd(nc, in_maps, core_ids=list(range(NCORES)))
    outT = np.concatenate([res.results[k]["out"] for k in range(NCORES)], axis=1)
    out = np.ascontiguousarray(outT.T, dtype=np.float32)  # [8192, 512]
    kernel.last_results = res
    return out



# revision 36
# speedup vs baseline: 1.0163x; 1.0163x over previous
"""Trainium2 kernel for nn_PolynomialLayer: out = [x, x_i*x_j (i<=j)] @ W.T + bias.

Data-parallel over batch across 8 NeuronCores. Each core:
  - receives x^T for its 1024-row batch shard ([128 feat, 1024 b]) plus 64
    partition-rotated copies (host np.roll; pure data movement),
  - builds the 8256 pairwise-product features on the vector engine as 65
    full-128-partition tensor_tensor multiplies (chunk d: xT * rot_d covers
    all index pairs with cyclic difference {d, 128-d}),
  - accumulates out^T[512, 1024] = sum_c Wc.T @ PTc on the tensor engine
    (66 K-chunks of 128, all 8 PSUM banks: 4 n-chunks x 2 b-chunks),
  - adds bias during the PSUM->SBUF copies (split scalar/vector engines).
The host pre-permutes/transposes the weight matrix so its column order
matches the on-chip feature-chunk layout.
"""

import os
import sys
import numpy as np

for _p in ("/opt/trn_rl_repo",):
    if os.path.isdir(_p) and _p not in sys.path:
        sys.path.append(_p)

B, D, NOUT = 8192, 128, 512
NCORES = 8
BC = B // NCORES            # 1024 batch rows per core
NCHUNK = 66                 # 1 linear + 1 squares + 64 rotation chunks
NROT = 64                   # rotation distances d=1..64
NB = BC // 512              # moving-operand chunks per core (2)
NN = NOUT // 128            # output partition chunks (4)

COMPUTE_DT = os.environ.get("POLY_COMPUTE_DT", "bfloat16")  # bfloat16 | mixed | float32r


def _ensure_axon_hooks_stub():
    """concourse's trace path imports antenv.axon_hooks; provide a stub if
    this image lacks it so an env-triggered trace degrades instead of
    crashing."""
    try:
        import antenv.axon_hooks  # noqa: F401
        return
    except Exception:
        pass
    try:
        import types
        import antenv
        m = types.ModuleType("antenv.axon_hooks")
        m._hook = None
        m.set_axon_ntff_profile_hook = lambda h: setattr(m, "_hook", h)
        m.get_axon_ntff_profile_hook = lambda: m._hook
        sys.modules["antenv.axon_hooks"] = m
        antenv.axon_hooks = m
    except Exception:
        pass


def _pair_index_map():
    """Map (chunk c, partition p) -> column index in the reference feature
    order (or -1 for padding).

    Reference order: [x_0..x_127] then pairs (i,j) i<=j in
    combinations_with_replacement order.
    Chunk layout: c=0 linear; c=1 squares; c=2..65 -> d=c-1 in 1..64 with
    (i,j) = sorted(p, (p+d) % 128); for d=64 only p<64 is valid.
    """
    idx = np.full((NCHUNK, D), -1, dtype=np.int64)
    off = 128 * np.arange(D) - (np.arange(D) * (np.arange(D) - 1)) // 2

    def pair_idx(i, j):
        return D + off[i] + (j - i)

    idx[0, :] = np.arange(D)
    p = np.arange(D)
    idx[1, :] = pair_idx(p, p)
    for d in range(1, NROT + 1):
        c = 1 + d
        q = (p + d) % D
        i = np.minimum(p, q)
        j = np.maximum(p, q)
        v = pair_idx(i, j)
        if d == NROT:
            v = np.where(p < 64, v, -1)
        idx[c, :] = v
    return idx


_nc_cache = None


def _build_nc():
    global _nc_cache
    if _nc_cache is not None:
        return _nc_cache
    import concourse.tile as tile
    from concourse import bacc, mybir

    # "mixed": f32r x/weights/products (precision), bf16 rotation streams (DMA)
    cdt = mybir.dt.float32r if COMPUTE_DT == "mixed" else getattr(mybir.dt, COMPUTE_DT)
    rdt = mybir.dt.bfloat16 if COMPUTE_DT == "mixed" else cdt
    nc = bacc.Bacc("TRN2", target_bir_lowering=False, debug=False)
    # partition-major DRAM layouts: one dma_start covers a GROUP of chunks
    # with large per-partition-contiguous descriptors.
    xT_ext = nc.dram_tensor("xT", [D, BC], cdt, kind="ExternalInput")
    rots_ext = nc.dram_tensor("rots", [D, NROT, BC], rdt, kind="ExternalInput")
    wp_ext = nc.dram_tensor("wp", [D, NCHUNK, NOUT], cdt, kind="ExternalInput")
    bias_ext = nc.dram_tensor("biasp", [D, NN], mybir.dt.float32, kind="ExternalInput")
    out_ext = nc.dram_tensor("out", [NOUT, BC], mybir.dt.float32, kind="ExternalOutput")

    # chunks per DMA group, small leading groups so the pipeline starts fast
    wg_sizes = [1, 1, 2, 4] + [6] * 9 + [4]          # sums to 66
    rg_sizes = [1, 1, 2, 4] + [6] * 9 + [2]          # sums to 64
    wg_starts = np.cumsum([0] + wg_sizes).tolist()
    rg_starts = np.cumsum([0] + rg_sizes).tolist()
    wg_of_chunk = {}
    for g, s in enumerate(wg_starts[:-1]):
        for c in range(s, wg_starts[g + 1]):
            wg_of_chunk[c] = g
    rg_of_rc = {}
    for g, s in enumerate(rg_starts[:-1]):
        for r in range(s, rg_starts[g + 1]):
            rg_of_rc[r] = g

    with tile.TileContext(nc) as tc:
        wide = COMPUTE_DT == "float32r"   # 4-byte rotations: tighter SBUF budget
        with (
            tc.tile_pool(name="xpool", bufs=1) as xpool,
            tc.tile_pool(name="wpool", bufs=4 if wide else 5) as wpool,
            tc.tile_pool(name="rpool", bufs=3 if wide else (4 if COMPUTE_DT == "mixed" else 6)) as rpool,
            tc.tile_pool(name="ppool", bufs=6 if COMPUTE_DT != "bfloat16" else 8) as ppool,
            tc.tile_pool(name="opool", bufs=1) as opool,
            tc.tile_pool(name="psum", bufs=1, space="PSUM") as psum,
        ):
            xT = xpool.tile([D, BC], cdt)
            nc.sync.dma_start(xT[:], xT_ext[:])

            ps = [[psum.tile([D, 512], mybir.dt.float32,
                             name=f"ps_{n}_{b}", tag=f"ps_{n}_{b}")
                   for b in range(NB)] for n in range(NN)]

            wg_tiles = {}
            rg_tiles = {}
            for c in range(NCHUNK):
                g = wg_of_chunk[c]
                if c == wg_starts[g]:
                    sz = wg_sizes[g]
                    wg = wpool.tile([D, sz * NOUT], cdt, name="wg", tag="wg")
                    nc.sync.dma_start(wg[:], wp_ext[:, c:c + sz, :])
                    wg_tiles[g] = wg
                rc = c - 2  # rotation index for this chunk
                if c >= 2:
                    rgi = rg_of_rc[rc]
                    if rc == rg_starts[rgi]:
                        sz = rg_sizes[rgi]
                        rg = rpool.tile([D, sz * BC], rdt, name="rg", tag="rg")
                        nc.sync.dma_start(rg[:], rots_ext[:, rc:rc + sz, :])
                        rg_tiles[rgi] = rg

                if c == 0:
                    mv = xT
                elif c == 1:
                    mv = ppool.tile([D, BC], cdt, name="pt", tag="pt")
                    nc.vector.tensor_mul(mv[:], xT[:], xT[:])
                else:
                    rg = rg_tiles[rg_of_rc[rc]]
                    roff = rc - rg_starts[rg_of_rc[rc]]
                    rslice = rg[:, roff * BC:(roff + 1) * BC]
                    mv = ppool.tile([D, BC], cdt, name="pt", tag="pt")
                    nc.vector.tensor_mul(mv[:], xT[:], rslice)
                wg = wg_tiles[g]
                woff = (c - wg_starts[g]) * NOUT
                for n in range(NN):
                    for b in range(NB):
                        nc.tensor.matmul(
                            ps[n][b][:],
                            wg[:, woff + n * 128:woff + (n + 1) * 128],
                            mv[:, b * 512:(b + 1) * 512],
                            start=(c == 0),
                            stop=(c == NCHUNK - 1),
                        )

            bias = xpool.tile([D, NN], mybir.dt.float32)
            nc.sync.dma_start(bias[:], bias_ext[:])
            obig = opool.tile([D, NN * NB * 512], mybir.dt.float32)
            for n in range(NN):
                for b in range(NB):
                    ot = obig[:, (n * NB + b) * 512:(n * NB + b + 1) * 512]
                    if b == 0:
                        nc.scalar.activation(
                            ot, ps[n][b][:],
                            mybir.ActivationFunctionType.Identity,
                            bias=bias[:, n:n + 1],
                        )
                    else:
                        nc.vector.tensor_scalar_add(ot, ps[n][b][:], bias[:, n:n + 1])
            # two halves so the first scatter overlaps the remaining copies
            h = NN // 2
            nc.sync.dma_start(
                out_ext[0:h * 128, :].rearrange("(n p) (b f) -> p n b f", n=h, b=NB),
                obig[:, 0:h * NB * 512].rearrange("p (n b f) -> p n b f", n=h, b=NB),
            )
            nc.sync.dma_start(
                out_ext[h * 128:, :].rearrange("(n p) (b f) -> p n b f", n=h, b=NB),
                obig[:, h * NB * 512:].rearrange("p (n b f) -> p n b f", n=h, b=NB),
            )

    nc.compile()
    _nc_cache = nc
    return nc


def _prep_inputs(x, weights, bias):
    if COMPUTE_DT == "bfloat16":
        import ml_dtypes
        cdt_np = np.dtype(ml_dtypes.bfloat16)
        rdt_np = cdt_np
    elif COMPUTE_DT == "mixed":
        import ml_dtypes
        cdt_np = np.dtype(np.float32)
        rdt_np = np.dtype(ml_dtypes.bfloat16)
    else:
        cdt_np = np.dtype(np.float32)
        rdt_np = cdt_np

    x = np.asarray(x, dtype=np.float32)
    weights = np.asarray(weights, dtype=np.float32)
    bias = np.asarray(bias, dtype=np.float32)

    idx = _pair_index_map()
    wcols = weights.T  # [8384, 512]
    wp = np.zeros((NCHUNK, D, NOUT), dtype=np.float32)
    valid = idx >= 0
    wp[valid] = wcols[idx[valid]]
    wp = np.ascontiguousarray(wp.transpose(1, 0, 2)).astype(cdt_np)  # [D, NCHUNK, NOUT]

    biasp = np.ascontiguousarray(bias.reshape(NN, 128).T)  # [128, NN] f32

    in_maps = []
    for k in range(NCORES):
        xs = np.ascontiguousarray(x[k * BC:(k + 1) * BC].T).astype(cdt_np)  # [128, BC]
        xr = xs.astype(rdt_np)
        rots = np.stack([np.roll(xr, -d, axis=0) for d in range(1, NROT + 1)])
        rots = rots.transpose(1, 0, 2)  # [D, NROT, BC] partition-major
        in_maps.append({
            "xT": xs,
            "rots": np.ascontiguousarray(rots),
            "wp": wp,
            "biasp": biasp,
        })
    return in_maps


def kernel(x, weights, bias):
    _ensure_axon_hooks_stub()
    from concourse.bass_utils import run_bass_kernel_spmd

    nc = _build_nc()
    in_maps = _prep_inputs(x, weights, bias)
    res = run_bass_kernel_spmd(nc, in_maps, core_ids=list(range(NCORES)))
    outT = np.concatenate([res.results[k]["out"] for k in range(NCORES)], axis=1)
    out = np.ascontiguousarray(outT.T, dtype=np.float32)  # [8192, 512]
    kernel.last_results = res
    return out

